# revision 8
# baseline (speedup 1.0000x reference)
"""Trainium2 Bass kernel for a 3-layer GCN (gnn_message_passing).

Strategy (8 NeuronCores, SPMD):
  - Nodes sharded contiguously: core c owns rows [c*NPC, (c+1)*NPC).
  - Per layer: h = dinv * (x @ W) computed locally (PE), written to DRAM,
    AllGather -> full h table; edges (grouped by dst-owning core, by
    128-node dst block, and by src< / >=32768 for int16 gather indices)
    are gathered 2048 rows at a time with dma_gather; a 0/1 selection
    matrix E (built with one is_equal op per chunk) turns the segment-sum
    into PSUM-accumulated matmuls h_gath.T @ E -> feature-major z.
  - BatchNorm: per-block column scale by dinv(dst), per-layer stats with
    ACT accumulate + a 1KB AllReduce, then one fused scale+shift+ReLU.
  - Head: PE transposes + one-hot graph-membership matmuls for the mean
    pool (SPMD-uniform), AllReduce, tiny head matmul.

All edge/index preprocessing happens on the host (it parameterizes the
compiled program); all floating-point math runs on device.
"""

import math

import numpy as np

_P = 128


def _ceil_div(a, b):
    return (a + b - 1) // b


def _wrap16(idx_i16):
    """[n] int16 -> [32, n/16]: element i at [i%16, i//16], replicated to
    rows 16..31 (the tx/rx Q7 cpus each read their own 16-partition copy)."""
    w = np.ascontiguousarray(idx_i16.reshape(-1, 16).T)
    return np.concatenate([w, w], axis=0)


class _Plan:
    """Host-side static schedule + per-core input arrays."""

    def __init__(self, n_nodes, n_edges, n_graphs, lo_split, chunk_tiles,
                 edge_index, batch, n_cores=8):
        P = _P
        self.n_nodes, self.n_graphs = n_nodes, n_graphs
        self.n_cores = n_cores
        self.npc = n_nodes // n_cores
        self.nb = _ceil_div(self.npc, P)
        self.npad = self.nb * P
        self.lo_split = lo_split
        self.ct = chunk_tiles

        src = np.asarray(edge_index[0], np.int64)
        dst = np.asarray(edge_index[1], np.int64)
        core_of = dst // self.npc
        local = dst - core_of * self.npc
        blk = local // P
        half = (src >= lo_split).astype(np.int64)
        gkey = (core_of * self.nb + blk) * 2 + half
        order = np.argsort(gkey, kind="stable")
        src_s, loc_s, gk_s = src[order], local[order], gkey[order]
        ngroups = n_cores * self.nb * 2
        counts = np.bincount(gk_s, minlength=ngroups)
        offs = np.concatenate([[0], np.cumsum(counts)])
        cnt3 = counts.reshape(n_cores, self.nb, 2)

        TL = np.maximum(
            _ceil_div(cnt3[:, :, 0].max(axis=0), P), 1)  # >=1: init PSUM
        TH = _ceil_div(cnt3[:, :, 1].max(axis=0), P)
        self.TL, self.TH = TL, TH
        self.SL = np.concatenate([[0], np.cumsum(TL)])
        self.SH = np.concatenate([[0], np.cumsum(TH)])
        TLtot, THtot = int(TL.sum()), int(TH.sum())
        self.CL = max(1, _ceil_div(TLtot, chunk_tiles))
        self.CH = max(0, _ceil_div(THtot, chunk_tiles)) if THtot else 0
        self.TLpad = self.CL * chunk_tiles
        self.THpad = max(self.CH * chunk_tiles, 1)

        idxL = np.zeros((n_cores, self.TLpad * P), np.int64)
        dstL = np.full((n_cores, self.TLpad * P), -1.0, np.float32)
        idxH = np.zeros((n_cores, self.THpad * P), np.int64)
        dstH = np.full((n_cores, self.THpad * P), -1.0, np.float32)
        for c in range(n_cores):
            for b in range(self.nb):
                for h, (S, idxA, dstA, base) in enumerate(
                        [(self.SL, idxL, dstL, 0),
                         (self.SH, idxH, dstH, lo_split)]):
                    g = (c * self.nb + b) * 2 + h
                    cnt = counts[g]
                    if cnt == 0:
                        continue
                    st = offs[g]
                    pos0 = int(S[b]) * P
                    idxA[c, pos0:pos0 + cnt] = src_s[st:st + cnt] - base
                    dstA[c, pos0:pos0 + cnt] = loc_s[st:st + cnt] - b * P
        assert idxL.max() < 32768 and idxH.max() < 32768
        self.idxL = np.stack([_wrap16(idxL[c].astype(np.int16))
                              for c in range(n_cores)])
        self.idxH = np.stack([_wrap16(idxH[c].astype(np.int16))
                              for c in range(n_cores)])
        # slot (j, t) = stream position t*128 + j
        self.dstL = np.ascontiguousarray(
            dstL.reshape(n_cores, self.TLpad, P).transpose(0, 2, 1))
        self.dstH = np.ascontiguousarray(
            dstH.reshape(n_cores, self.THpad, P).transpose(0, 2, 1))

        # degree (f32 counts; float math happens on device)
        deg = np.bincount(dst, minlength=n_nodes).astype(np.float32)
        degp = np.zeros((n_cores, self.npad), np.float32)
        for c in range(n_cores):
            degp[c, :self.npc] = deg[c * self.npc:(c + 1) * self.npc]
        self.deg_nm = np.ascontiguousarray(
            degp.reshape(n_cores, self.nb, P).transpose(0, 2, 1))
        self.deg_flat = degp[:, None, :]  # [cores, 1, npad]

        batch = np.asarray(batch, np.int64)
        bv = np.full((n_cores, self.npad), -1.0, np.float32)
        for c in range(n_cores):
            bv[c, :self.npc] = batch[c * self.npc:(c + 1) * self.npc]
        self.batchv = np.ascontiguousarray(
            bv.reshape(n_cores, self.nb, P).transpose(0, 2, 1))
        self.cnt_g = np.bincount(batch, minlength=n_graphs).astype(
            np.float32)[:, None]  # [G, 1]

    def schedule_key(self):
        return (self.n_nodes, self.n_cores, self.lo_split, self.ct,
                tuple(self.TL.tolist()), tuple(self.TH.tolist()))


def _build(plan, hid, out_c, eps, use_bf16):
    import concourse.bacc as bacc
    import concourse.tile as tile
    import concourse.mybir as mybir
    from concourse.masks import make_identity

    P, D = _P, hid
    NB, NPAD, NPC = plan.nb, plan.npad, plan.npc
    CT, CL, CH = plan.ct, plan.CL, plan.CH
    G = plan.n_graphs
    f32 = mybir.dt.float32
    hdt = mybir.dt.bfloat16 if use_bf16 else f32
    AF = mybir.ActivationFunctionType
    OP = mybir.AluOpType

    nc = bacc.Bacc("TRN2", target_bir_lowering=False, debug=False,
                   num_devices=plan.n_cores)

    # ---- I/O ----
    x_in = nc.dram_tensor("x_fm", [P, NPAD], f32, kind="ExternalInput")
    W_in = [nc.dram_tensor(f"W{l}", [D, D], f32, kind="ExternalInput")
            for l in range(3)]
    g_in = [nc.dram_tensor(f"g{l}", [D, 1], f32, kind="ExternalInput")
            for l in range(3)]
    be_in = [nc.dram_tensor(f"be{l}", [D, 1], f32, kind="ExternalInput")
             for l in range(3)]
    Wh_in = nc.dram_tensor("Wh", [D, out_c], f32, kind="ExternalInput")
    bh_in = nc.dram_tensor("bh_mat", [G, out_c], f32, kind="ExternalInput")
    cnt_in = nc.dram_tensor("cnt_g", [G, 1], f32, kind="ExternalInput")
    degnm_in = nc.dram_tensor("deg_nm", [P, NB], f32, kind="ExternalInput")
    degfl_in = nc.dram_tensor("deg_flat", [1, NPAD], f32,
                              kind="ExternalInput")
    bv_in = nc.dram_tensor("batchv", [P, NB], f32, kind="ExternalInput")
    idxL_in = nc.dram_tensor("idxL", [32, plan.TLpad * P // 16],
                             mybir.dt.int16, kind="ExternalInput")
    idxH_in = nc.dram_tensor("idxH", [32, plan.THpad * P // 16],
                             mybir.dt.int16, kind="ExternalInput")
    dstL_in = nc.dram_tensor("dstL", [P, plan.TLpad], f32,
                             kind="ExternalInput")
    dstH_in = nc.dram_tensor("dstH", [P, plan.THpad], f32,
                             kind="ExternalInput")
    out_t = nc.dram_tensor("out", [G, out_c], f32, kind="ExternalOutput")

    rg = [list(range(plan.n_cores))]
    n_slab = _ceil_div(NPAD, 512)

    with tile.TileContext(nc) as tc:
        with (
            tc.tile_pool(name="per", bufs=1) as per,      # persistent
            tc.tile_pool(name="gb", bufs=2) as gb,        # gather chunk bufs
            tc.tile_pool(name="eb", bufs=2) as eb,        # E chunk bufs
            tc.tile_pool(name="wk", bufs=3) as wk,        # small working tiles
            tc.tile_pool(name="ps", bufs=2, space="PSUM") as ps,
            tc.tile_pool(name="psa", bufs=3, space="PSUM") as psa,
            tc.tile_pool(name="ps1", bufs=1, space="PSUM") as ps1,
            tc.tile_pool(name="dram", bufs=1, space="DRAM") as dram,
        ):
            # ---------------- prologue: constants / metadata ----------------
            x_fm = per.tile([P, NPAD], f32)
            nc.sync.dma_start(out=x_fm[:], in_=x_in[:, :])
            z_fm = per.tile([P, NPAD], f32)
            dinv_bc = per.tile([P, NPAD], f32)
            HG = 8  # transform staging: blocks per DMA group

            W_sb = [per.tile([D, D], f32, name=f"W{l}_sb") for l in range(3)]
            g_sb = [per.tile([D, 1], f32, name=f"g{l}_sb") for l in range(3)]
            be_sb = [per.tile([D, 1], f32, name=f"be{l}_sb")
                     for l in range(3)]
            for l in range(3):
                nc.sync.dma_start(out=W_sb[l][:], in_=W_in[l][:, :])
                nc.sync.dma_start(out=g_sb[l][:], in_=g_in[l][:, :])
                nc.sync.dma_start(out=be_sb[l][:], in_=be_in[l][:, :])
            Wh_sb = per.tile([D, out_c], f32)
            nc.sync.dma_start(out=Wh_sb[:], in_=Wh_in[:, :])
            bh_sb = per.tile([G, out_c], f32)
            nc.sync.dma_start(out=bh_sb[:], in_=bh_in[:, :])
            cnt_sb = per.tile([G, 1], f32)
            nc.sync.dma_start(out=cnt_sb[:], in_=cnt_in[:, :])
            bv_sb = per.tile([P, NB], f32)
            nc.sync.dma_start(out=bv_sb[:], in_=bv_in[:, :])
            idxL_sb = per.tile([32, plan.TLpad * P // 16], mybir.dt.int16)
            nc.sync.dma_start(out=idxL_sb[:], in_=idxL_in[:, :])
            idxH_sb = per.tile([32, plan.THpad * P // 16], mybir.dt.int16)
            nc.sync.dma_start(out=idxH_sb[:], in_=idxH_in[:, :])
            dstL_sb = per.tile([P, plan.TLpad], f32)
            nc.sync.dma_start(out=dstL_sb[:], in_=dstL_in[:, :])
            dstH_sb = per.tile([P, plan.THpad], f32)
            nc.sync.dma_start(out=dstH_sb[:], in_=dstH_in[:, :])

            ident = per.tile([P, P], f32)
            make_identity(nc, ident[:])

            iota_wide = per.tile([P, CT * P], f32)
            nc.gpsimd.iota(iota_wide[:], pattern=[[0, CT], [1, P]],
                           base=0, channel_multiplier=0,
                           allow_small_or_imprecise_dtypes=True)
            iotaG = per.tile([P, G], f32)
            nc.gpsimd.iota(iotaG[:], pattern=[[1, G]], base=0,
                           channel_multiplier=0,
                           allow_small_or_imprecise_dtypes=True)

            def dinv_ops(t_ap, out_ap, sc_pool, shape, nm):
                """out = where(deg>0, rsqrt(max(deg,1)), 0) elementwise."""
                mx = sc_pool.tile(shape, f32, tag=f"dsc_a", name=f"mx_{nm}")
                nc.vector.tensor_scalar_max(mx[:], t_ap, 1.0)
                rec = sc_pool.tile(shape, f32, tag=f"dsc_b", name=f"rec_{nm}")
                nc.vector.reciprocal(rec[:], mx[:])
                sq = sc_pool.tile(shape, f32, tag=f"dsc_a", name=f"sq_{nm}")
                nc.scalar.sqrt(sq[:], rec[:])
                mask = sc_pool.tile(shape, f32, tag=f"dsc_b",
                                    name=f"mask_{nm}")
                nc.vector.tensor_scalar(mask[:], t_ap, 0.0, None,
                                        op0=OP.is_gt)
                nc.vector.tensor_tensor(out=out_ap, in0=sq[:], in1=mask[:],
                                        op=OP.mult)

            degnm_sb = per.tile([P, NB], f32)
            nc.sync.dma_start(out=degnm_sb[:], in_=degnm_in[:, :])
            dinv_nm = per.tile([P, NB], f32)
            dinv_ops(degnm_sb[:], dinv_nm[:], wk, [P, NB], "nm")

            ones_row = per.tile([1, P], f32)
            nc.vector.memset(ones_row[:], 1.0)
            for s in range(n_slab):
                w = min(512, NPAD - 512 * s)
                dr = wk.tile([1, 512], f32, tag="drow")
                nc.sync.dma_start(out=dr[:, :w],
                                  in_=degfl_in[:, 512 * s:512 * s + w])
                bc_ps = ps1.tile([P, 512], f32, tag="bc")
                nc.tensor.matmul(out=bc_ps[:, :w], lhsT=ones_row[:],
                                 rhs=dr[:, :w], start=True, stop=True)
                degbc = wk.tile([P, 512], f32, tag="scr512")
                nc.scalar.copy(degbc[:, :w], bc_ps[:, :w])
                dinv_ops(degbc[:, :w], dinv_bc[:, 512 * s:512 * s + w],
                         wk, [P, w], f"bc{s}")

            # DRAM scratch
            pool_l = dram.tile([P, G], f32)
            pool_g = dram.tile([P, G], f32, addr_space="Shared")

            n_full_blk = NPC // P          # blocks with all 128 rows valid
            rem = NPC - n_full_blk * P     # rows in the last partial block

            # ---------------- layers ----------------
            for l in range(3):
                h_local = dram.tile([NPC, D], hdt, name=f"h_local_{l}")
                h_full = dram.tile([plan.n_nodes, D], hdt,
                                   addr_space="Shared", name=f"h_full_{l}")
                stats_l = dram.tile([P, 2], f32, name=f"stats_l_{l}")
                stats_g = dram.tile([P, 2], f32, addr_space="Shared",
                                    name=f"stats_g_{l}")
                # transform: h = dinv * (x @ W), node-major, staged per HG
                for g0 in range(0, NB, HG):
                    g1 = min(g0 + HG, NB)
                    st = gb.tile([P, HG, D], hdt, tag="hstage",
                                 name=f"hstage_{l}_{g0}")
                    for i in range(g0, g1):
                        h_ps = ps.tile([P, D], f32, tag="hps")
                        nc.tensor.matmul(out=h_ps[:],
                                         lhsT=x_fm[:, P * i:P * (i + 1)],
                                         rhs=W_sb[l][:],
                                         start=True, stop=True)
                        nc.scalar.activation(out=st[:, i - g0, :],
                                             in_=h_ps[:], func=AF.Copy,
                                             scale=dinv_nm[:, i:i + 1])
                    nfull = min(g1, n_full_blk) - g0
                    if nfull > 0:
                        nc.sync.dma_start(
                            out=h_local[g0 * P:(g0 + nfull) * P, :].rearrange(
                                "(t p) f -> p t f", p=P),
                            in_=st[:, 0:nfull, :])
                    if rem and g1 == NB:
                        nc.sync.dma_start(
                            out=h_local[n_full_blk * P:NPC, :],
                            in_=st[:rem, n_full_blk - g0, :])
                nc.gpsimd.collective_compute(
                    "AllGather", OP.bypass, replica_groups=rg,
                    ins=[h_local.opt()], outs=[h_full.opt()])

                # gather + aggregate
                h_lo = h_full[0:plan.lo_split, :]
                h_hi = h_full[plan.lo_split:plan.n_nodes, :]
                chunks = {}

                def ensure_chunk(stream, ci, l=l, chunks=chunks):
                    key = (stream, ci)
                    if key in chunks:
                        return chunks[key]
                    src_ap = h_lo if stream == "L" else h_hi
                    idxs = idxL_sb if stream == "L" else idxH_sb
                    dsts = dstL_sb if stream == "L" else dstH_sb
                    buf = gb.tile([P, CT, D], hdt, tag=f"gb{stream}",
                                  name=f"gb{stream}_{l}_{ci}")
                    nc.gpsimd.dma_gather(
                        out_ap=buf[:],
                        in_ap=src_ap,
                        idxs_ap=idxs[:, ci * CT * P // 16:
                                     (ci + 1) * CT * P // 16],
                        num_idxs=CT * P, num_idxs_reg=CT * P, elem_size=D,
                        single_packet=False)
                    E = eb.tile([P, CT, P], hdt, tag=f"eb{stream}",
                                name=f"eb{stream}_{l}_{ci}")
                    nc.vector.tensor_tensor(
                        out=E[:],
                        in0=iota_wide[:].rearrange("p (t c) -> p t c", c=P),
                        in1=dsts[:, ci * CT:(ci + 1) * CT].to_broadcast(
                            [P, CT, P]),
                        op=OP.is_equal)
                    chunks[key] = (buf, E)
                    return chunks[key]

                for b in range(NB):
                    acc = psa.tile([D, P], f32, tag="acc")
                    total = int(plan.TL[b] + plan.TH[b])
                    k = 0
                    for stream, T, S in (("L", plan.TL, plan.SL),
                                         ("H", plan.TH, plan.SH)):
                        for t in range(int(T[b])):
                            pos = int(S[b]) + t
                            ci, sl = pos // CT, pos % CT
                            buf, E = ensure_chunk(stream, ci)
                            nc.tensor.matmul(
                                out=acc[:], lhsT=buf[:, sl, :],
                                rhs=E[:, sl, :],
                                start=(k == 0), stop=(k == total - 1))
                            k += 1
                    nc.vector.tensor_tensor(
                        out=z_fm[:, P * b:P * (b + 1)], in0=acc[:],
                        in1=dinv_bc[:, P * b:P * (b + 1)], op=OP.mult)

                # stats: sum and sum-of-squares over all columns
                stats_sb = wk.tile([P, 2], f32, tag="stats")
                nc.vector.reduce_sum(out=stats_sb[:, 0:1], in_=z_fm[:],
                                     axis=mybir.AxisListType.X)
                sqacc = wk.tile([P, n_slab], f32, tag="sqacc")
                for s in range(n_slab):
                    w = min(512, NPAD - 512 * s)
                    scr = wk.tile([P, 512], f32, tag="sqscr")
                    nc.scalar.activation(
                        out=scr[:, :w], in_=z_fm[:, 512 * s:512 * s + w],
                        func=AF.Square, accum_out=sqacc[:, s:s + 1])
                nc.vector.reduce_sum(out=stats_sb[:, 1:2], in_=sqacc[:],
                                     axis=mybir.AxisListType.X)
                nc.sync.dma_start(out=stats_l[:], in_=stats_sb[:])
                nc.gpsimd.collective_compute(
                    "AllReduce", OP.add, replica_groups=rg,
                    ins=[stats_l.opt()], outs=[stats_g.opt()])
                gstats = wk.tile([P, 2], f32, tag="gstats")
                nc.sync.dma_start(out=gstats[:], in_=stats_g[:])

                inv_n = 1.0 / plan.n_nodes
                mu = wk.tile([P, 1], f32, tag="mu")
                nc.vector.tensor_scalar_mul(mu[:], gstats[:, 0:1], inv_n)
                ex2 = wk.tile([P, 1], f32, tag="ex2")
                nc.vector.tensor_scalar_mul(ex2[:], gstats[:, 1:2], inv_n)
                musq = wk.tile([P, 1], f32, tag="musq")
                nc.vector.tensor_tensor(out=musq[:], in0=mu[:], in1=mu[:],
                                        op=OP.mult)
                var = wk.tile([P, 1], f32, tag="var")
                nc.vector.tensor_tensor(out=var[:], in0=ex2[:], in1=musq[:],
                                        op=OP.subtract)
                vpe = wk.tile([P, 1], f32, tag="vpe")
                nc.vector.tensor_scalar_add(vpe[:], var[:], eps)
                rec = wk.tile([P, 1], f32, tag="rec")
                nc.vector.reciprocal(rec[:], vpe[:])
                rs = wk.tile([P, 1], f32, tag="rs")
                nc.scalar.sqrt(rs[:], rec[:])
                scale = wk.tile([P, 1], f32, tag="scale")
                nc.vector.tensor_tensor(out=scale[:], in0=rs[:],
                                        in1=g_sb[l][:], op=OP.mult)
                mus = wk.tile([P, 1], f32, tag="mus")
                nc.vector.tensor_tensor(out=mus[:], in0=mu[:],
                                        in1=scale[:], op=OP.mult)
                shift = wk.tile([P, 1], f32, tag="shift")
                nc.vector.tensor_tensor(out=shift[:], in0=be_sb[l][:],
                                        in1=mus[:], op=OP.subtract)
                nc.scalar.activation(out=x_fm[:], in_=z_fm[:], func=AF.Relu,
                                     bias=shift[:], scale=scale[:])

            # ---------------- global mean pool + head ----------------
            pool_ps = ps1.tile([D, G], f32, tag="poolps")
            for i in range(NB):
                tp_ps = ps.tile([P, P], f32, tag="hps")
                nc.tensor.transpose(out=tp_ps[:],
                                    in_=x_fm[:, P * i:P * (i + 1)],
                                    identity=ident[:])
                xt = wk.tile([P, P], f32, tag="xt")
                nc.scalar.copy(xt[:], tp_ps[:])
                B = wk.tile([P, G], f32, tag="B")
                nc.vector.tensor_scalar(B[:], iotaG[:], bv_sb[:, i:i + 1],
                                        None, op0=OP.is_equal)
                nc.tensor.matmul(out=pool_ps[:], lhsT=xt[:], rhs=B[:],
                                 start=(i == 0), stop=(i == NB - 1))
            pool_sb = wk.tile([P, G], f32, tag="poolsb")
            nc.scalar.copy(pool_sb[:], pool_ps[:])
            nc.sync.dma_start(out=pool_l[:], in_=pool_sb[:])
            nc.gpsimd.collective_compute(
                "AllReduce", OP.add, replica_groups=rg,
                ins=[pool_l.opt()], outs=[pool_g.opt()])
            pool_full = wk.tile([P, G], f32, tag="poolfull")
            nc.sync.dma_start(out=pool_full[:], in_=pool_g[:])

            cmax = wk.tile([G, 1], f32, tag="cmax")
            nc.vector.tensor_scalar_max(cmax[:], cnt_sb[:], 1.0)
            cinv = wk.tile([G, 1], f32, tag="cinv")
            nc.vector.reciprocal(cinv[:], cmax[:])

            head_ps = ps1.tile([G, out_c], f32, tag="headps")
            nc.tensor.matmul(out=head_ps[:], lhsT=pool_full[:],
                             rhs=Wh_sb[:], start=True, stop=True)
            head_sb = wk.tile([G, out_c], f32, tag="headsb")
            nc.scalar.activation(out=head_sb[:], in_=head_ps[:],
                                 func=AF.Copy, scale=cinv[:])
            head_fin = wk.tile([G, out_c], f32, tag="headfin")
            nc.vector.tensor_tensor(out=head_fin[:], in0=head_sb[:],
                                    in1=bh_sb[:], op=OP.add)
            nc.sync.dma_start(out=out_t[:, :], in_=head_fin[:])

    nc.compile()
    return nc


_BUILD_CACHE = {}


def _get_built(plan, hid, out_c, eps, use_bf16):
    key = plan.schedule_key() + (hid, out_c, use_bf16)
    if key not in _BUILD_CACHE:
        _BUILD_CACHE[key] = _build(plan, hid, out_c, eps, use_bf16)
    return _BUILD_CACHE[key]


def _make_in_maps(plan, inputs, hid):
    x = np.asarray(inputs["x"], np.float32)
    Ws = {f"W{l}": np.ascontiguousarray(
        np.asarray(inputs[f"W{l}"], np.float32)) for l in range(3)}
    gs = {f"g{l}": np.asarray(
        inputs[f"g{l}"], np.float32).reshape(hid, 1) for l in range(3)}
    bes = {f"be{l}": np.asarray(
        inputs[f"be{l}"], np.float32).reshape(hid, 1) for l in range(3)}
    Wh = np.ascontiguousarray(np.asarray(inputs["Wh"], np.float32))
    bh = np.asarray(inputs["bh"], np.float32)
    bh_mat = np.ascontiguousarray(
        np.tile(bh[None, :], (plan.n_graphs, 1)).astype(np.float32))

    in_maps = []
    for c in range(plan.n_cores):
        xs = np.zeros((_P, plan.npad), np.float32)
        xs[:, :plan.npc] = x[c * plan.npc:(c + 1) * plan.npc].T
        m = {
            "x_fm": xs,
            "Wh": Wh, "bh_mat": bh_mat, "cnt_g": plan.cnt_g,
            "deg_nm": plan.deg_nm[c], "deg_flat": plan.deg_flat[c],
            "batchv": plan.batchv[c],
            "idxL": plan.idxL[c], "idxH": plan.idxH[c],
            "dstL": plan.dstL[c], "dstH": plan.dstH[c],
        }
        m.update(Ws)
        m.update(gs)
        m.update(bes)
        in_maps.append(m)
    return in_maps


def run(inputs, n_nodes=50000, n_graphs=64, hid=128, out_c=8,
        lo_split=32768, chunk_tiles=16, eps=1e-5, use_bf16=False,
        n_cores=8):
    import concourse.bass_utils as bass_utils

    edge_index = np.asarray(inputs["edge_index"], np.int64)
    plan = _Plan(n_nodes, edge_index.shape[1], n_graphs, lo_split,
                 chunk_tiles, edge_index, inputs["batch"], n_cores)
    nc = _get_built(plan, hid, out_c, eps, use_bf16)
    in_maps = _make_in_maps(plan, inputs, hid)
    res = bass_utils.run_bass_kernel_spmd(
        nc, in_maps, core_ids=list(range(n_cores)))
    return np.asarray(res.results[0]["out"], np.float32)


def kernel(**inputs) -> np.ndarray:
    return run(inputs)


# revision 12
# speedup vs baseline: 2.0642x; 2.0642x over previous
"""Trainium2 Bass kernel for a 3-layer GCN (gnn_message_passing).

Strategy (8 NeuronCores, SPMD):
  - Nodes sharded contiguously: core c owns rows [c*NPC, (c+1)*NPC).
  - Per layer: h = dinv * (x @ W) computed locally (PE), written to DRAM,
    AllGather -> full h table; edges (grouped by dst-owning core, by
    128-node dst block, and by src< / >=32768 for int16 gather indices)
    are gathered 2048 rows at a time with dma_gather; a 0/1 selection
    matrix E (built with one is_equal op per chunk) turns the segment-sum
    into PSUM-accumulated matmuls h_gath.T @ E -> feature-major z.
  - BatchNorm: per-block column scale by dinv(dst), per-layer stats with
    ACT accumulate + a 1KB AllReduce, then one fused scale+shift+ReLU.
  - Head: PE transposes + one-hot graph-membership matmuls for the mean
    pool (SPMD-uniform), AllReduce, tiny head matmul.

All edge/index preprocessing happens on the host (it parameterizes the
compiled program); all floating-point math runs on device.
"""

import math

import numpy as np

_P = 128


def _ceil_div(a, b):
    return (a + b - 1) // b


def _wrap16(idx_i16):
    """[n] int16 -> [32, n/16]: element i at [i%16, i//16], replicated to
    rows 16..31 (the tx/rx Q7 cpus each read their own 16-partition copy)."""
    w = np.ascontiguousarray(idx_i16.reshape(-1, 16).T)
    return np.concatenate([w, w], axis=0)




def _f32_layout(plan):
    P, D, G, NB, NPAD = _P, 128, plan.n_graphs, plan.nb, plan.npad
    sizes = [
        ("x_fm", P * NPAD), ("W0", D * D), ("W1", D * D), ("W2", D * D),
        ("g0", D), ("g1", D), ("g2", D),
        ("be0", D), ("be1", D), ("be2", D),
        ("Wh", D * 8), ("bh_mat", G * 8), ("cnt_g", G),
        ("deg_nm", P * NB), ("deg_flat", NPAD), ("batchv", P * NB),
        ("dstL", P * plan.TLpad), ("dstH", P * plan.THpad),
    ]
    off, cur = {}, 0
    for k, n in sizes:
        off[k] = cur
        cur += n
    return off, cur


def _i16_layout(plan):
    P = _P
    sizes = [("idxL", 32 * plan.TLpad * P // 16),
             ("idxH", 32 * plan.THpad * P // 16)]
    off, cur = {}, 0
    for k, n in sizes:
        off[k] = cur
        cur += n
    return off, cur


class _Plan:
    """Host-side static schedule + per-core input arrays."""

    def __init__(self, n_nodes, n_edges, n_graphs, lo_split, chunk_tiles,
                 edge_index, batch, n_cores=8):
        P = _P
        self.n_nodes, self.n_graphs = n_nodes, n_graphs
        self.n_cores = n_cores
        self.npc = n_nodes // n_cores
        self.nb = _ceil_div(self.npc, P)
        self.npad = self.nb * P
        self.lo_split = lo_split
        self.ct = chunk_tiles

        src = np.asarray(edge_index[0], np.int64)
        dst = np.asarray(edge_index[1], np.int64)
        core_of = dst // self.npc
        local = dst - core_of * self.npc
        blk = local // P
        half = (src >= lo_split).astype(np.int64)
        gkey = (core_of * self.nb + blk) * 2 + half
        order = np.argsort(gkey, kind="stable")
        src_s, loc_s, gk_s = src[order], local[order], gkey[order]
        ngroups = n_cores * self.nb * 2
        counts = np.bincount(gk_s, minlength=ngroups)
        offs = np.concatenate([[0], np.cumsum(counts)])
        cnt3 = counts.reshape(n_cores, self.nb, 2)

        TL = np.maximum(
            _ceil_div(cnt3[:, :, 0].max(axis=0), P), 1)  # >=1: init PSUM
        TH = _ceil_div(cnt3[:, :, 1].max(axis=0), P)
        self.TL, self.TH = TL, TH
        self.SL = np.concatenate([[0], np.cumsum(TL)])
        self.SH = np.concatenate([[0], np.cumsum(TH)])
        TLtot, THtot = int(TL.sum()), int(TH.sum())
        self.CL = max(1, _ceil_div(TLtot, chunk_tiles))
        self.CH = max(0, _ceil_div(THtot, chunk_tiles)) if THtot else 0
        self.TLpad = self.CL * chunk_tiles
        self.THpad = max(self.CH * chunk_tiles, 1)

        idxL = np.zeros((n_cores, self.TLpad * P), np.int64)
        dstL = np.full((n_cores, self.TLpad * P), -1.0, np.float32)
        idxH = np.zeros((n_cores, self.THpad * P), np.int64)
        dstH = np.full((n_cores, self.THpad * P), -1.0, np.float32)
        for c in range(n_cores):
            for b in range(self.nb):
                for h, (S, idxA, dstA, base) in enumerate(
                        [(self.SL, idxL, dstL, 0),
                         (self.SH, idxH, dstH, lo_split)]):
                    g = (c * self.nb + b) * 2 + h
                    cnt = counts[g]
                    if cnt == 0:
                        continue
                    st = offs[g]
                    pos0 = int(S[b]) * P
                    idxA[c, pos0:pos0 + cnt] = src_s[st:st + cnt] - base
                    dstA[c, pos0:pos0 + cnt] = loc_s[st:st + cnt] - b * P
        assert idxL.max() < 32768 and idxH.max() < 32768
        self.idxL = np.stack([_wrap16(idxL[c].astype(np.int16))
                              for c in range(n_cores)])
        self.idxH = np.stack([_wrap16(idxH[c].astype(np.int16))
                              for c in range(n_cores)])
        # slot (j, t) = stream position t*128 + j
        self.dstL = np.ascontiguousarray(
            dstL.reshape(n_cores, self.TLpad, P).transpose(0, 2, 1))
        self.dstH = np.ascontiguousarray(
            dstH.reshape(n_cores, self.THpad, P).transpose(0, 2, 1))

        # degree (f32 counts; float math happens on device)
        deg = np.bincount(dst, minlength=n_nodes).astype(np.float32)
        degp = np.zeros((n_cores, self.npad), np.float32)
        for c in range(n_cores):
            degp[c, :self.npc] = deg[c * self.npc:(c + 1) * self.npc]
        self.deg_nm = np.ascontiguousarray(
            degp.reshape(n_cores, self.nb, P).transpose(0, 2, 1))
        self.deg_flat = degp[:, None, :]  # [cores, 1, npad]

        batch = np.asarray(batch, np.int64)
        bv = np.full((n_cores, self.npad), -1.0, np.float32)
        for c in range(n_cores):
            bv[c, :self.npc] = batch[c * self.npc:(c + 1) * self.npc]
        self.batchv = np.ascontiguousarray(
            bv.reshape(n_cores, self.nb, P).transpose(0, 2, 1))
        self.cnt_g = np.bincount(batch, minlength=n_graphs).astype(
            np.float32)[:, None]  # [G, 1]

    def schedule_key(self):
        return (self.n_nodes, self.n_cores, self.lo_split, self.ct,
                tuple(self.TL.tolist()), tuple(self.TH.tolist()))


def _build(plan, hid, out_c, eps, use_bf16, mode="full"):
    import concourse.bacc as bacc
    import concourse.tile as tile
    import concourse.mybir as mybir
    from concourse.masks import make_identity

    P, D = _P, hid
    NB, NPAD, NPC = plan.nb, plan.npad, plan.npc
    CT, CL, CH = plan.ct, plan.CL, plan.CH
    G = plan.n_graphs
    f32 = mybir.dt.float32
    hdt = mybir.dt.bfloat16 if use_bf16 else f32
    AF = mybir.ActivationFunctionType
    OP = mybir.AluOpType

    nc = bacc.Bacc("TRN2", target_bir_lowering=False, debug=False,
                   num_devices=plan.n_cores)

    # ---- I/O: two packed blobs (each input tensor costs ~1.3ms/exec on
    # the axon PJRT path, so everything is packed into pf (f32) + pi (i16)
    # and viewed via APs; device-side DMA behavior is unchanged) ----
    foff, flen = _f32_layout(plan)
    pf_t = nc.dram_tensor("pf", [flen], f32, kind="ExternalInput")
    ioff, ilen = _i16_layout(plan)
    pi_t = nc.dram_tensor("pi", [ilen], mybir.dt.int16, kind="ExternalInput")

    def fv(name, p, c):  # 2-D partition-major view [p, c]
        o = foff[name]
        return pf_t[o:o + p * c].rearrange("(p c) -> p c", c=c)

    x_in = fv("x_fm", P, NPAD)
    W_in = [fv(f"W{l}", D, D) for l in range(3)]
    g_in = [fv(f"g{l}", D, 1) for l in range(3)]
    be_in = [fv(f"be{l}", D, 1) for l in range(3)]
    Wh_in = fv("Wh", D, out_c)
    bh_in = fv("bh_mat", G, out_c)
    cnt_in = fv("cnt_g", G, 1)
    degnm_in = fv("deg_nm", P, NB)
    degfl_in = fv("deg_flat", 1, NPAD)
    bv_in = fv("batchv", P, NB)
    dstL_in = fv("dstL", P, plan.TLpad)
    dstH_in = fv("dstH", P, plan.THpad)

    def iv(name, p, c):
        o = ioff[name]
        return pi_t[o:o + p * c].rearrange("(p c) -> p c", c=c)

    idxL_in = iv("idxL", 32, plan.TLpad * P // 16)
    idxH_in = iv("idxH", 32, plan.THpad * P // 16)
    out_t = nc.dram_tensor("out", [G, out_c], f32, kind="ExternalOutput")

    rg = [list(range(plan.n_cores))]
    n_slab = _ceil_div(NPAD, 512)

    with tile.TileContext(nc) as tc:
        with (
            tc.tile_pool(name="per", bufs=1) as per,      # persistent
            tc.tile_pool(name="gb", bufs=2) as gb,        # gather chunk bufs
            tc.tile_pool(name="eb", bufs=2) as eb,        # E chunk bufs
            tc.tile_pool(name="wk", bufs=3) as wk,        # small working tiles
            tc.tile_pool(name="ps", bufs=2, space="PSUM") as ps,
            tc.tile_pool(name="psa", bufs=3, space="PSUM") as psa,
            tc.tile_pool(name="ps1", bufs=1, space="PSUM") as ps1,
            tc.tile_pool(name="dram", bufs=1, space="DRAM") as dram,
        ):
            # ---------------- prologue: constants / metadata ----------------
            x_fm = per.tile([P, NPAD], f32)
            nc.sync.dma_start(out=x_fm[:], in_=x_in)
            z_fm = per.tile([P, NPAD], f32)
            dinv_bc = per.tile([P, NPAD], f32)
            HG = 8  # transform staging: blocks per DMA group

            W_sb = [per.tile([D, D], f32, name=f"W{l}_sb") for l in range(3)]
            g_sb = [per.tile([D, 1], f32, name=f"g{l}_sb") for l in range(3)]
            be_sb = [per.tile([D, 1], f32, name=f"be{l}_sb")
                     for l in range(3)]
            for l in range(3):
                nc.sync.dma_start(out=W_sb[l][:], in_=W_in[l])
                nc.sync.dma_start(out=g_sb[l][:], in_=g_in[l])
                nc.sync.dma_start(out=be_sb[l][:], in_=be_in[l])
            Wh_sb = per.tile([D, out_c], f32)
            nc.sync.dma_start(out=Wh_sb[:], in_=Wh_in)
            bh_sb = per.tile([G, out_c], f32)
            nc.sync.dma_start(out=bh_sb[:], in_=bh_in)
            cnt_sb = per.tile([G, 1], f32)
            nc.sync.dma_start(out=cnt_sb[:], in_=cnt_in)
            bv_sb = per.tile([P, NB], f32)
            nc.sync.dma_start(out=bv_sb[:], in_=bv_in)
            idxL_sb = per.tile([32, plan.TLpad * P // 16], mybir.dt.int16)
            nc.sync.dma_start(out=idxL_sb[:], in_=idxL_in)
            idxH_sb = per.tile([32, plan.THpad * P // 16], mybir.dt.int16)
            nc.sync.dma_start(out=idxH_sb[:], in_=idxH_in)
            dstL_sb = per.tile([P, plan.TLpad], f32)
            nc.sync.dma_start(out=dstL_sb[:], in_=dstL_in)
            dstH_sb = per.tile([P, plan.THpad], f32)
            nc.sync.dma_start(out=dstH_sb[:], in_=dstH_in)

            ident = per.tile([P, P], f32)
            make_identity(nc, ident[:])

            iota_wide = per.tile([P, CT * P], f32)
            nc.gpsimd.iota(iota_wide[:], pattern=[[0, CT], [1, P]],
                           base=0, channel_multiplier=0,
                           allow_small_or_imprecise_dtypes=True)
            iotaG = per.tile([P, G], f32)
            nc.gpsimd.iota(iotaG[:], pattern=[[1, G]], base=0,
                           channel_multiplier=0,
                           allow_small_or_imprecise_dtypes=True)

            def dinv_ops(t_ap, out_ap, sc_pool, shape, nm):
                """out = where(deg>0, rsqrt(max(deg,1)), 0) elementwise."""
                mx = sc_pool.tile(shape, f32, tag=f"dsc_a", name=f"mx_{nm}")
                nc.vector.tensor_scalar_max(mx[:], t_ap, 1.0)
                rec = sc_pool.tile(shape, f32, tag=f"dsc_b", name=f"rec_{nm}")
                nc.vector.reciprocal(rec[:], mx[:])
                sq = sc_pool.tile(shape, f32, tag=f"dsc_a", name=f"sq_{nm}")
                nc.scalar.sqrt(sq[:], rec[:])
                mask = sc_pool.tile(shape, f32, tag=f"dsc_b",
                                    name=f"mask_{nm}")
                nc.vector.tensor_scalar(mask[:], t_ap, 0.0, None,
                                        op0=OP.is_gt)
                nc.vector.tensor_tensor(out=out_ap, in0=sq[:], in1=mask[:],
                                        op=OP.mult)

            degnm_sb = per.tile([P, NB], f32)
            nc.sync.dma_start(out=degnm_sb[:], in_=degnm_in)
            dinv_nm = per.tile([P, NB], f32)
            dinv_ops(degnm_sb[:], dinv_nm[:], wk, [P, NB], "nm")

            ones_row = per.tile([1, P], f32)
            nc.vector.memset(ones_row[:], 1.0)
            for s in range(n_slab):
                w = min(512, NPAD - 512 * s)
                dr = wk.tile([1, 512], f32, tag="drow")
                nc.sync.dma_start(out=dr[:, :w],
                                  in_=degfl_in[:, 512 * s:512 * s + w])
                bc_ps = ps1.tile([P, 512], f32, tag="bc")
                nc.tensor.matmul(out=bc_ps[:, :w], lhsT=ones_row[:],
                                 rhs=dr[:, :w], start=True, stop=True)
                degbc = wk.tile([P, 512], f32, tag="scr512")
                nc.scalar.copy(degbc[:, :w], bc_ps[:, :w])
                dinv_ops(degbc[:, :w], dinv_bc[:, 512 * s:512 * s + w],
                         wk, [P, w], f"bc{s}")

            # DRAM scratch
            pool_l = dram.tile([P, G], f32)
            pool_g = dram.tile([P, G], f32, addr_space="Shared")

            n_full_blk = NPC // P          # blocks with all 128 rows valid
            rem = NPC - n_full_blk * P     # rows in the last partial block

            # ---------------- layers ----------------
            layer_range = 0 if mode == "empty" else 3
            for l in range(layer_range):
                h_local = dram.tile([NPC, D], hdt, name=f"h_local_{l}")
                h_full = dram.tile([plan.n_nodes, D], hdt,
                                   addr_space="Shared", name=f"h_full_{l}")
                stats_l = dram.tile([P, 2], f32, name=f"stats_l_{l}")
                stats_g = dram.tile([P, 2], f32, addr_space="Shared",
                                    name=f"stats_g_{l}")
                # transform: h = dinv * (x @ W), node-major, staged per HG
                for g0 in range(0, NB, HG):
                    g1 = min(g0 + HG, NB)
                    st = gb.tile([P, HG, D], hdt, tag="hstage",
                                 name=f"hstage_{l}_{g0}")
                    for i in range(g0, g1):
                        h_ps = ps.tile([P, D], f32, tag="hps")
                        nc.tensor.matmul(out=h_ps[:],
                                         lhsT=x_fm[:, P * i:P * (i + 1)],
                                         rhs=W_sb[l][:],
                                         start=True, stop=True)
                        nc.scalar.activation(out=st[:, i - g0, :],
                                             in_=h_ps[:], func=AF.Copy,
                                             scale=dinv_nm[:, i:i + 1])
                    nfull = min(g1, n_full_blk) - g0
                    if nfull > 0:
                        nc.sync.dma_start(
                            out=h_local[g0 * P:(g0 + nfull) * P, :].rearrange(
                                "(t p) f -> p t f", p=P),
                            in_=st[:, 0:nfull, :])
                    if rem and g1 == NB:
                        nc.sync.dma_start(
                            out=h_local[n_full_blk * P:NPC, :],
                            in_=st[:rem, n_full_blk - g0, :])
                if mode not in ("xform",):
                    nc.gpsimd.collective_compute(
                        "AllGather", OP.bypass, replica_groups=rg,
                        ins=[h_local.opt()], outs=[h_full.opt()])

                # gather + aggregate
                h_lo = h_full[0:plan.lo_split, :]
                h_hi = h_full[plan.lo_split:plan.n_nodes, :]
                chunks = {}

                def ensure_chunk(stream, ci, l=l, chunks=chunks):
                    key = (stream, ci)
                    if key in chunks:
                        return chunks[key]
                    src_ap = h_lo if stream == "L" else h_hi
                    idxs = idxL_sb if stream == "L" else idxH_sb
                    dsts = dstL_sb if stream == "L" else dstH_sb
                    buf = gb.tile([P, CT, D], hdt, tag=f"gb{stream}",
                                  name=f"gb{stream}_{l}_{ci}")
                    if mode != "nogather":
                        nc.gpsimd.dma_gather(
                            out_ap=buf[:],
                            in_ap=src_ap,
                            idxs_ap=idxs[:, ci * CT * P // 16:
                                         (ci + 1) * CT * P // 16],
                            num_idxs=CT * P, num_idxs_reg=CT * P,
                            elem_size=D, single_packet=False)
                    E = eb.tile([P, CT, P], hdt, tag=f"eb{stream}",
                                name=f"eb{stream}_{l}_{ci}")
                    nc.vector.tensor_tensor(
                        out=E[:],
                        in0=iota_wide[:].rearrange("p (t c) -> p t c", c=P),
                        in1=dsts[:, ci * CT:(ci + 1) * CT].to_broadcast(
                            [P, CT, P]),
                        op=OP.is_equal)
                    chunks[key] = (buf, E)
                    return chunks[key]

                if mode in ("noagg", "xform", "xform_ag"):
                    nc.vector.memset(z_fm[:], 0.0)
                for b in range(NB if mode == "full" else 0):
                    acc = psa.tile([D, P], f32, tag="acc")
                    total = int(plan.TL[b] + plan.TH[b])
                    k = 0
                    for stream, T, S in (("L", plan.TL, plan.SL),
                                         ("H", plan.TH, plan.SH)):
                        for t in range(int(T[b])):
                            pos = int(S[b]) + t
                            ci, sl = pos // CT, pos % CT
                            buf, E = ensure_chunk(stream, ci)
                            nc.tensor.matmul(
                                out=acc[:], lhsT=buf[:, sl, :],
                                rhs=E[:, sl, :],
                                start=(k == 0), stop=(k == total - 1))
                            k += 1
                    nc.vector.tensor_tensor(
                        out=z_fm[:, P * b:P * (b + 1)], in0=acc[:],
                        in1=dinv_bc[:, P * b:P * (b + 1)], op=OP.mult)

                # stats: sum and sum-of-squares over all columns
                if mode in ("xform", "xform_ag"):
                    continue
                stats_sb = wk.tile([P, 2], f32, tag="stats")
                nc.vector.reduce_sum(out=stats_sb[:, 0:1], in_=z_fm[:],
                                     axis=mybir.AxisListType.X)
                sqacc = wk.tile([P, n_slab], f32, tag="sqacc")
                for s in range(n_slab):
                    w = min(512, NPAD - 512 * s)
                    scr = wk.tile([P, 512], f32, tag="sqscr")
                    nc.scalar.activation(
                        out=scr[:, :w], in_=z_fm[:, 512 * s:512 * s + w],
                        func=AF.Square, accum_out=sqacc[:, s:s + 1])
                nc.vector.reduce_sum(out=stats_sb[:, 1:2], in_=sqacc[:],
                                     axis=mybir.AxisListType.X)
                nc.sync.dma_start(out=stats_l[:], in_=stats_sb[:])
                nc.gpsimd.collective_compute(
                    "AllReduce", OP.add, replica_groups=rg,
                    ins=[stats_l.opt()], outs=[stats_g.opt()])
                gstats = wk.tile([P, 2], f32, tag="gstats")
                nc.sync.dma_start(out=gstats[:], in_=stats_g[:])

                inv_n = 1.0 / plan.n_nodes
                mu = wk.tile([P, 1], f32, tag="mu")
                nc.vector.tensor_scalar_mul(mu[:], gstats[:, 0:1], inv_n)
                ex2 = wk.tile([P, 1], f32, tag="ex2")
                nc.vector.tensor_scalar_mul(ex2[:], gstats[:, 1:2], inv_n)
                musq = wk.tile([P, 1], f32, tag="musq")
                nc.vector.tensor_tensor(out=musq[:], in0=mu[:], in1=mu[:],
                                        op=OP.mult)
                var = wk.tile([P, 1], f32, tag="var")
                nc.vector.tensor_tensor(out=var[:], in0=ex2[:], in1=musq[:],
                                        op=OP.subtract)
                vpe = wk.tile([P, 1], f32, tag="vpe")
                nc.vector.tensor_scalar_add(vpe[:], var[:], eps)
                rec = wk.tile([P, 1], f32, tag="rec")
                nc.vector.reciprocal(rec[:], vpe[:])
                rs = wk.tile([P, 1], f32, tag="rs")
                nc.scalar.sqrt(rs[:], rec[:])
                scale = wk.tile([P, 1], f32, tag="scale")
                nc.vector.tensor_tensor(out=scale[:], in0=rs[:],
                                        in1=g_sb[l][:], op=OP.mult)
                mus = wk.tile([P, 1], f32, tag="mus")
                nc.vector.tensor_tensor(out=mus[:], in0=mu[:],
                                        in1=scale[:], op=OP.mult)
                shift = wk.tile([P, 1], f32, tag="shift")
                nc.vector.tensor_tensor(out=shift[:], in0=be_sb[l][:],
                                        in1=mus[:], op=OP.subtract)
                nc.scalar.activation(out=x_fm[:], in_=z_fm[:], func=AF.Relu,
                                     bias=shift[:], scale=scale[:])

            # ---------------- global mean pool + head ----------------
            do_pool = mode in ("full", "noagg")
            pool_ps = ps1.tile([D, G], f32, tag="poolps")
            for i in range(NB if do_pool else 1):
                tp_ps = ps.tile([P, P], f32, tag="hps")
                nc.tensor.transpose(out=tp_ps[:],
                                    in_=x_fm[:, P * i:P * (i + 1)],
                                    identity=ident[:])
                xt = wk.tile([P, P], f32, tag="xt")
                nc.scalar.copy(xt[:], tp_ps[:])
                B = wk.tile([P, G], f32, tag="B")
                nc.vector.tensor_scalar(B[:], iotaG[:], bv_sb[:, i:i + 1],
                                        None, op0=OP.is_equal)
                nc.tensor.matmul(out=pool_ps[:], lhsT=xt[:], rhs=B[:],
                                 start=(i == 0),
                                 stop=(i == (NB if do_pool else 1) - 1))
            pool_sb = wk.tile([P, G], f32, tag="poolsb")
            nc.scalar.copy(pool_sb[:], pool_ps[:])
            nc.sync.dma_start(out=pool_l[:], in_=pool_sb[:])
            nc.gpsimd.collective_compute(
                "AllReduce", OP.add, replica_groups=rg,
                ins=[pool_l.opt()], outs=[pool_g.opt()])
            pool_full = wk.tile([P, G], f32, tag="poolfull")
            nc.sync.dma_start(out=pool_full[:], in_=pool_g[:])

            cmax = wk.tile([G, 1], f32, tag="cmax")
            nc.vector.tensor_scalar_max(cmax[:], cnt_sb[:], 1.0)
            cinv = wk.tile([G, 1], f32, tag="cinv")
            nc.vector.reciprocal(cinv[:], cmax[:])

            head_ps = ps1.tile([G, out_c], f32, tag="headps")
            nc.tensor.matmul(out=head_ps[:], lhsT=pool_full[:],
                             rhs=Wh_sb[:], start=True, stop=True)
            head_sb = wk.tile([G, out_c], f32, tag="headsb")
            nc.scalar.activation(out=head_sb[:], in_=head_ps[:],
                                 func=AF.Copy, scale=cinv[:])
            head_fin = wk.tile([G, out_c], f32, tag="headfin")
            nc.vector.tensor_tensor(out=head_fin[:], in0=head_sb[:],
                                    in1=bh_sb[:], op=OP.add)
            nc.sync.dma_start(out=out_t[:, :], in_=head_fin[:])

    nc.compile()
    return nc


_BUILD_CACHE = {}


def _get_built(plan, hid, out_c, eps, use_bf16, mode="full"):
    key = plan.schedule_key() + (hid, out_c, use_bf16, mode)
    if key not in _BUILD_CACHE:
        _BUILD_CACHE[key] = _build(plan, hid, out_c, eps, use_bf16, mode)
    return _BUILD_CACHE[key]


def _make_in_maps(plan, inputs, hid):
    P, G = _P, plan.n_graphs
    x = np.asarray(inputs["x"], np.float32)
    bh = np.asarray(inputs["bh"], np.float32)
    bh_mat = np.tile(bh[None, :], (G, 1)).astype(np.float32)
    foff, flen = _f32_layout(plan)
    ioff, ilen = _i16_layout(plan)

    in_maps = []
    for c in range(plan.n_cores):
        pf = np.zeros(flen, np.float32)

        def put(name, arr):
            a = np.ascontiguousarray(np.asarray(arr, np.float32))
            pf[foff[name]:foff[name] + a.size] = a.ravel()

        xs = np.zeros((P, plan.npad), np.float32)
        xs[:, :plan.npc] = x[c * plan.npc:(c + 1) * plan.npc].T
        put("x_fm", xs)
        for l in range(3):
            put(f"W{l}", inputs[f"W{l}"])
            put(f"g{l}", inputs[f"g{l}"])
            put(f"be{l}", inputs[f"be{l}"])
        put("Wh", inputs["Wh"])
        put("bh_mat", bh_mat)
        put("cnt_g", plan.cnt_g)
        put("deg_nm", plan.deg_nm[c])
        put("deg_flat", plan.deg_flat[c])
        put("batchv", plan.batchv[c])
        put("dstL", plan.dstL[c])
        put("dstH", plan.dstH[c])

        pi = np.zeros(ilen, np.int16)
        pi[ioff["idxL"]:ioff["idxL"] + plan.idxL[c].size] = \
            plan.idxL[c].ravel()
        pi[ioff["idxH"]:ioff["idxH"] + plan.idxH[c].size] = \
            plan.idxH[c].ravel()
        in_maps.append({"pf": pf, "pi": pi})
    return in_maps


def _exec_sig(nc):
    import concourse.mybir as mybir
    partition_name = (nc.partition_id_tensor.name
                      if nc.partition_id_tensor else None)
    in_names, out_names, out_avals, zero_outs = [], [], [], []
    import jax
    for alloc in nc.m.functions[0].allocations:
        if not isinstance(alloc, mybir.MemoryLocationSet):
            continue
        name = alloc.memorylocations[0].name
        if alloc.kind == "ExternalInput":
            if name != partition_name:
                in_names.append(name)
        elif alloc.kind == "ExternalOutput":
            shape = tuple(alloc.tensor_shape)
            dtype = mybir.dt.np(alloc.dtype)
            out_names.append(name)
            out_avals.append(jax.core.ShapedArray(shape, dtype))
            zero_outs.append(np.zeros(shape, dtype))
    return partition_name, in_names, out_names, out_avals, zero_outs


_FN_CACHE = {}


def _get_exec_fn(nc, n_cores):
    """One jitted shard_map'd bass_exec call over n_cores devices,
    cached so repeated kernel() calls skip retracing."""
    key = (id(nc), n_cores)
    if key in _FN_CACHE:
        return _FN_CACHE[key]
    import jax
    from jax.sharding import Mesh, PartitionSpec
    from jax.experimental.shard_map import shard_map
    from concourse.bass2jax import (_bass_exec_p, install_neuronx_cc_hook,
                                    partition_id_tensor)

    install_neuronx_cc_hook()
    partition_name, in_names, out_names, out_avals, zero_outs = _exec_sig(nc)
    n_params = len(in_names)
    n_outs = len(out_avals)
    all_in_names = tuple(in_names + out_names +
                         ([partition_name] if partition_name else []))

    def _body(*args):
        operands = list(args)
        if partition_name:
            operands.append(partition_id_tensor())
        return tuple(_bass_exec_p.bind(
            *operands,
            out_avals=tuple(out_avals),
            in_names=all_in_names,
            out_names=tuple(out_names),
            lowering_input_output_aliases=(),
            sim_require_finite=True,
            sim_require_nnan=True,
            nc=nc,
        ))

    donate = tuple(range(n_params, n_params + n_outs))
    devices = jax.devices()[:n_cores]
    mesh = Mesh(np.asarray(devices), ("core",))
    in_specs = (PartitionSpec("core"),) * (n_params + n_outs)
    out_specs = (PartitionSpec("core"),) * n_outs
    fn = jax.jit(
        shard_map(_body, mesh=mesh, in_specs=in_specs,
                  out_specs=out_specs, check_rep=False),
        donate_argnums=donate, keep_unused=True)
    meta = (in_names, out_names, out_avals, zero_outs, n_params)
    _FN_CACHE[key] = (fn, meta)
    return _FN_CACHE[key]


def _execute(nc, in_maps, n_cores):
    fn, (in_names, out_names, out_avals, zero_outs, _np_) = _get_exec_fn(
        nc, n_cores)
    concat_in = [np.concatenate([np.asarray(m[name]) for m in in_maps],
                                axis=0) for name in in_names]
    concat_zeros = [np.zeros((n_cores * z.shape[0], *z.shape[1:]), z.dtype)
                    for z in zero_outs]
    out_arrs = fn(*concat_in, *concat_zeros)
    return [
        {name: np.asarray(out_arrs[i]).reshape(
            n_cores, *out_avals[i].shape)[c]
         for i, name in enumerate(out_names)}
        for c in range(n_cores)
    ]


_PLAN_CACHE = {}


def _get_plan(n_nodes, n_graphs, lo_split, chunk_tiles, edge_index, batch,
              n_cores):
    key = (n_nodes, lo_split, chunk_tiles, n_cores,
           hash(edge_index.tobytes()), hash(np.asarray(batch).tobytes()))
    if key not in _PLAN_CACHE:
        _PLAN_CACHE[key] = _Plan(n_nodes, edge_index.shape[1], n_graphs,
                                 lo_split, chunk_tiles, edge_index, batch,
                                 n_cores)
    return _PLAN_CACHE[key]


def run(inputs, n_nodes=50000, n_graphs=64, hid=128, out_c=8,
        lo_split=32768, chunk_tiles=16, eps=1e-5, use_bf16=False,
        n_cores=8, mode="full"):
    edge_index = np.asarray(inputs["edge_index"], np.int64)
    plan = _get_plan(n_nodes, n_graphs, lo_split, chunk_tiles, edge_index,
                     inputs["batch"], n_cores)
    nc = _get_built(plan, hid, out_c, eps, use_bf16, mode)
    in_maps = _make_in_maps(plan, inputs, hid)
    results = _execute(nc, in_maps, n_cores)
    return np.asarray(results[0]["out"], np.float32)


def kernel(**inputs) -> np.ndarray:
    return run(inputs)
